# revision 5
# baseline (speedup 1.0000x reference)
"""Trainium2 Bass kernel for nn_PosMLP (box-relative position MLP).

Math (per query n):
  weights = q @ Wg + bg                       [N, 65]
  w1x = weights[:, 0:16], w1y = weights[:, 16:32]
  b1  = weights[:, 32:48], w2  = weights[:, 48:64], b2 = weights[:, 64]
  rel_x[j] = (xs[j] - cx)/bw ; rel_y[i] = (ys[i] - cy)/bh
  pre[i,j,k] = rel_x[j]*w1x[k] + rel_y[i]*w1y[k] + b1[k]
  out[i,j] = sum_k relu(pre)[i,j,k] * w2[k] + b2           (64x64 image)

Device mapping (per core, N=300 queries padded to 304 = 38 blocks of 8):
  - fold pos into per-(n,k) affine coeffs: ax = w1x/bw, ay = w1y/bh,
    c = b1 - cx*ax - cy*ay, so pre = xs[j]*ax + ys[i]*ay + c.
  - create: PE matmul K=3: lhsT = Wpre[3, 128(n,k)] per block,
    rhs = G[3, 512 pixels] (xs/ys/1 rows) -> psum pre[128, 512].
  - relu: ACT/DVE psum->SBUF.
  - contract: PE matmul K=128 with zero-padded block-diagonal
    lhsT[128(n,k), 128 n-cols]; 16 blocks accumulate into ONE psum bank
    -> dense [128 n, 512 pix] output; + b2 via per-partition ts-add.
All matmuls in float32r (full-speed fp32 path, ~3e-4 rel err).
"""
import numpy as np
from contextlib import ExitStack

# ---------------- problem constants (hardcoded per spec) ----------------
B_, Q_, DIM = 8, 300, 256
HD = 16            # hidden_dim
H = W = 64
NPIX = H * W       # 4096
CH = 512           # pixels per chunk
NCH = NPIX // CH   # 8
NCORE = 8
NPER = (B_ * Q_) // NCORE     # 300 queries per core
NPAD = 304                    # padded to 38 blocks of 8
NBLK = NPAD // 8              # 38
# psum groups: 16 blocks = 128 n rows per group (last group 6 blocks / 48 rows)
GROUPS = [(0, 16, 128), (16, 16, 128), (32, 6, 48)]
CGRP = 4                      # chunks per chunk-group (psum out banks)

_CACHE = {}


def _install_patches():
    """Shims for this container: walrus here rejects >1 sem-wait per
    instruction; split extra waits onto same-engine NOP carriers."""
    import bass_rust
    import concourse.tile as tile

    def _drain_and_barrier(self, tick_clock, wait_clock):
        nc = self.nc
        probe = nc.sync.nop(nofuse=True, hint="tile_drain_probe")
        wait_clock.add_sem_waits(
            probe.ins, bass_rust.ScopedClock({None: tick_clock.global_clock})
        )
        waits = list(probe.ins.sync_info.on_wait)
        probe.ins.sync_info.on_wait = []
        for i, w in enumerate(waits):
            carrier = nc.sync.nop(nofuse=True, hint=f"tile_drain_wait{i}")
            si = carrier.ins.sync_info
            if si is None:
                si = bass_rust.SyncInfo(on_wait=[], on_update=[])
                carrier.ins.sync_info = si
            si.on_wait = [w]
        nc.sync.drain()
        nc.all_engine_barrier()
        assert self.sems is not None
        popped = nc._tile_sem_poison_stack.pop()
        assert popped is self._sem_poison
        nc.clear_and_free_semaphores(list(self.sems.allocated().values()))
        nc.all_engine_barrier()

    tile.TileContext._drain_and_barrier = _drain_and_barrier


def _split_waits(nc, max_waits=1):
    import concourse.mybir as mybir

    for f in nc.m.functions:
        for bb in f.blocks:
            insts = bb.instructions
            out = []
            changed = False
            for inst in insts:
                si = inst.sync_info
                waits = list(si.on_wait) if si is not None else []
                if len(waits) > max_waits:
                    keep = waits[:max_waits]
                    rest = waits[max_waits:]
                    for j in range(0, len(rest), max_waits):
                        out.append(
                            mybir.InstNoOp(
                                name=f"{inst.name}_wsplit{j}",
                                engine=inst.engine,
                                bass_nofuse=True,
                                sync_info=mybir.SyncInfo(
                                    on_wait=rest[j : j + max_waits], on_update=[]
                                ),
                            )
                        )
                    si.on_wait = keep
                    changed = True
                out.append(inst)
            if changed:
                insts[:] = out


def _build():
    import concourse.bass as bass
    import concourse.tile as tile
    import concourse.mybir as mybir
    from concourse.masks import make_identity

    _install_patches()
    f32 = mybir.dt.float32
    f32r = mybir.dt.float32r
    AO = mybir.AluOpType
    Relu = mybir.ActivationFunctionType.Relu

    nc = bass.Bass()
    qT_t = nc.dram_tensor("qT", (DIM, NPAD), f32, kind="ExternalInput")
    pos_t = nc.dram_tensor("posn", (NPAD, 4), f32, kind="ExternalInput")
    wg_t = nc.dram_tensor("wg", (DIM, 4 * HD + 1), f32, kind="ExternalInput")
    bg_t = nc.dram_tensor("bgr", (1, 4 * HD + 1), f32, kind="ExternalInput")
    g_t = nc.dram_tensor("gx", (3, NPIX), f32r, kind="ExternalInput")
    out_t = nc.dram_tensor("out", (NPAD, NPIX), f32, kind="ExternalOutput")

    NC65 = 4 * HD + 1

    with ExitStack() as ctx:
        tc = ctx.enter_context(tile.TileContext(nc))
        sb = ctx.enter_context(tc.tile_pool(name="sb", bufs=1))

        # ---------------- load inputs ----------------
        qt0 = sb.tile([128, NPAD], f32, tag="qt0")
        nc.sync.dma_start(qt0[:], qT_t[0:128, :])
        qt1 = sb.tile([128, NPAD], f32, tag="qt1")
        nc.sync.dma_start(qt1[:], qT_t[128:256, :])
        wg0 = sb.tile([128, NC65], f32, tag="wg0")
        nc.sync.dma_start(wg0[:], wg_t[0:128, :])
        wg1 = sb.tile([128, NC65], f32, tag="wg1")
        nc.sync.dma_start(wg1[:], wg_t[128:256, :])
        bgr = sb.tile([1, NC65], f32, tag="bgr")
        nc.sync.dma_start(bgr[:], bg_t[:])
        posn = sb.tile([128, 12], f32, tag="posn")
        g_sb = sb.tile([3, NPIX], f32r, tag="gsb")
        nc.sync.dma_start(g_sb[:], g_t[:])

        ones1 = sb.tile([1, 128], f32, tag="ones1")
        nc.vector.memset(ones1[:], 1.0)
        ident = sb.tile([128, 128], f32, tag="ident")
        make_identity(nc, ident)

        inv = sb.tile([128, 6], f32, tag="inv")
        wn = sb.tile([128, 3 * NC65], f32, tag="wn")
        axn = sb.tile([128, 48], f32, tag="axn")
        ayn = sb.tile([128, 48], f32, tag="ayn")
        cn = sb.tile([128, 48], f32, tag="cn")
        tmp = sb.tile([128, 48], f32, tag="tmp")
        b2n = sb.tile([128, 3], f32, tag="b2n")
        w2t = sb.tile([HD, NPAD], f32, tag="w2t")
        wpre = sb.tile([3, NPAD * HD], f32r, tag="wpre")
        w2big = sb.tile([128, NBLK * 128], f32r, tag="w2big")
        nc.gpsimd.memset(w2big[:].bitcast(f32), 0.0)

        with tc.tile_pool(name="setps", bufs=2, space="PSUM") as setps:
            for ci, (r0, nr) in enumerate([(0, 128), (128, 128), (256, 48)]):
                nc.sync.dma_start(posn[0:nr, 4 * ci:4 * ci + 4], pos_t[r0:r0 + nr, :])
                nc.vector.reciprocal(inv[0:nr, 2 * ci:2 * ci + 2],
                                     posn[0:nr, 4 * ci + 2:4 * ci + 4])
                # weights by-n = qT_chunk.T @ Wg + bg
                wps = setps.tile([128, NC65], f32, tag="wps")
                nc.tensor.matmul(wps[0:nr, :], qt0[:, r0:r0 + nr], wg0[:],
                                 start=True, stop=False)
                nc.tensor.matmul(wps[0:nr, :], qt1[:, r0:r0 + nr], wg1[:],
                                 start=False, stop=False)
                nc.tensor.matmul(wps[0:nr, :], ones1[:, 0:nr], bgr[:],
                                 start=False, stop=True)
                wnc = wn[0:nr, NC65 * ci:NC65 * ci + NC65]
                nc.vector.tensor_copy(wnc, wps[0:nr, :])

                axc = axn[0:nr, 16 * ci:16 * ci + 16]
                ayc = ayn[0:nr, 16 * ci:16 * ci + 16]
                cc = cn[0:nr, 16 * ci:16 * ci + 16]
                tc_ = tmp[0:nr, 16 * ci:16 * ci + 16]
                ibw = inv[0:nr, 2 * ci:2 * ci + 1]
                ibh = inv[0:nr, 2 * ci + 1:2 * ci + 2]
                cx = posn[0:nr, 4 * ci:4 * ci + 1]
                cy = posn[0:nr, 4 * ci + 1:4 * ci + 2]
                nc.vector.tensor_scalar(axc, wnc[:, 0:16], ibw, None, AO.mult)
                nc.vector.tensor_scalar(ayc, wnc[:, 16:32], ibh, None, AO.mult)
                nc.vector.tensor_scalar(tc_, axc, cx, None, AO.mult)
                nc.vector.tensor_sub(cc, wnc[:, 32:48], tc_)
                nc.vector.tensor_scalar(tc_, ayc, cy, None, AO.mult)
                nc.vector.tensor_sub(cc, cc, tc_)
                nc.scalar.copy(b2n[0:nr, ci:ci + 1], wnc[:, 64:65])

                # w2 (by-n) -> transposed [16, n]
                wtp = setps.tile([HD, 128], f32, tag="wtp")
                nc.tensor.transpose(wtp[:, 0:nr], wnc[:, 48:64], ident[0:nr, 0:nr])
                nc.vector.tensor_copy(w2t[:, r0:r0 + nr], wtp[:, 0:nr])

                # Wpre rows: partition->free flatten  [nr, 16] -> [1, nr*16]
                for r, src in ((0, axc), (1, ayc), (2, cc)):
                    dst = wpre[r:r + 1, HD * r0: HD * r0 + HD * nr]
                    nc.sync.dma_start(
                        dst.bitcast(f32).rearrange("p (n k) -> p n k", k=HD), src)

            # w2big: zero-padded block-diagonal lhsT per block
            # col(B, m) = 128*B + 8*(B%16) + m ; partition rows 16m..16m+16
            # B = 16g + b -> col = 2048*g + 136*b + m
            for m in range(8):
                t = w2big[16 * m:16 * m + 16, 0:1].tensor
                ts_ = w2t[:, 0:1].tensor
                for g0, nb in ((0, 16), (1, 16), (2, 6)):
                    dst = bass.AP(t, (16 * m) * (NBLK * 128) + 2048 * g0 + m,
                                  [[NBLK * 128, 16], [136, nb]]).bitcast(f32)
                    src = bass.AP(ts_, 128 * g0 + m,
                                  [[NPAD, 16], [8, nb]])
                    nc.sync.dma_start(dst, src)

        # ---------------- main loop ----------------
        pre_pool = ctx.enter_context(tc.tile_pool(name="prep", bufs=3, space="PSUM"))
        out_pool = ctx.enter_context(tc.tile_pool(name="outp", bufs=4, space="PSUM"))
        act_pool = ctx.enter_context(tc.tile_pool(name="actp", bufs=6))
        osb_pool = ctx.enter_context(tc.tile_pool(name="osbp", bufs=3))

        rl = 0  # relu parity for ACT/DVE balance
        for (b0, nblk, nrows) in GROUPS:
            gbase = 8 * b0
            for cg in range(NCH // CGRP):
                outs = [out_pool.tile([128, CH], f32, tag="ops", name=f"ops{cg}_{i}")
                        for i in range(CGRP)]
                for b in range(nblk):
                    Bb = b0 + b
                    acts = []
                    for ci in range(CGRP):
                        c = CGRP * cg + ci
                        pre = pre_pool.tile([128, CH], f32, tag="pre")
                        nc.tensor.matmul(pre[:],
                                         wpre[:, 128 * Bb:128 * Bb + 128],
                                         g_sb[:, CH * c:CH * c + CH],
                                         start=True, stop=True)
                        act = act_pool.tile([128, CH], f32r, tag="act")
                        if rl % 2 == 0:
                            nc.scalar.activation(act[:], pre[:], Relu)
                        else:
                            nc.vector.tensor_scalar(act[:], pre[:], 0.0, None, AO.max)
                        rl += 1
                        acts.append(act)
                    for ci in range(CGRP):
                        nc.tensor.matmul(outs[ci][0:nrows, :],
                                         w2big[:, 128 * Bb:128 * Bb + 128][:, 0:nrows],
                                         acts[ci][:],
                                         start=(b == 0), stop=(b == nblk - 1),
                                         skip_group_check=True)
                for ci in range(CGRP):
                    c = CGRP * cg + ci
                    osb = osb_pool.tile([128, CH], f32, tag="osb")
                    gi = b0 // 16
                    nc.scalar.add(osb[0:nrows, :], outs[ci][0:nrows, :],
                                  b2n[0:nrows, gi:gi + 1])
                    nc.sync.dma_start(out_t[gbase:gbase + nrows, CH * c:CH * c + CH],
                                      osb[0:nrows, :])

    _split_waits(nc)
    return nc


def _get_nc():
    if "nc" not in _CACHE:
        _CACHE["nc"] = _build()
    return _CACHE["nc"]


def _prepare_in_maps(pos, queries, Wg, bg):
    pos_f = np.asarray(pos, np.float32).reshape(-1, 4)
    q_f = np.asarray(queries, np.float32).reshape(-1, DIM)
    Wg = np.ascontiguousarray(np.asarray(Wg, np.float32))
    bg = np.ascontiguousarray(np.asarray(bg, np.float32).reshape(1, 4 * HD + 1))

    xs = (np.arange(W, dtype=np.float32) + 0.5) / W
    ys = (np.arange(H, dtype=np.float32) + 0.5) / H
    gx = np.ascontiguousarray(
        np.stack([np.tile(xs, H), np.repeat(ys, W),
                  np.ones(NPIX, np.float32)]).astype(np.float32))

    in_maps = []
    for ci in range(NCORE):
        rows = slice(NPER * ci, NPER * (ci + 1))
        qT = np.zeros((DIM, NPAD), np.float32)
        qT[:, :NPER] = q_f[rows].T
        posn = np.tile(np.array([0.5, 0.5, 1.0, 1.0], np.float32), (NPAD, 1))
        posn[:NPER] = pos_f[rows]
        in_maps.append({"qT": qT, "posn": posn, "wg": Wg, "bgr": bg, "gx": gx})
    return in_maps


def kernel(pos, queries, Wg, bg, size):
    from concourse.bass_utils import run_bass_kernel_spmd

    assert int(size) == 64
    in_maps = _prepare_in_maps(pos, queries, Wg, bg)
    res = run_bass_kernel_spmd(_get_nc(), in_maps, list(range(NCORE)), trace=False)
    outs = [res.results[i]["out"][:NPER].reshape(NPER, H, W) for i in range(NCORE)]
    return np.concatenate(outs, 0).reshape(B_, Q_, H, W).astype(np.float32)


# revision 7
# speedup vs baseline: 1.2147x; 1.2147x over previous
"""Trainium2 Bass kernel for nn_PosMLP (box-relative position MLP).

Math (per query n):
  weights = q @ Wg + bg                       [N, 65]
  w1x = weights[:, 0:16], w1y = weights[:, 16:32]
  b1  = weights[:, 32:48], w2  = weights[:, 48:64], b2 = weights[:, 64]
  rel_x[j] = (xs[j] - cx)/bw ; rel_y[i] = (ys[i] - cy)/bh
  pre[i,j,k] = rel_x[j]*w1x[k] + rel_y[i]*w1y[k] + b1[k]
  out[i,j] = sum_k relu(pre)[i,j,k] * w2[k] + b2           (64x64 image)

Device mapping (per core, N=300 queries padded to 304 = 38 blocks of 8):
  - fold pos into per-(n,k) affine coeffs: ax = w1x/bw, ay = w1y/bh,
    c = b1 - cx*ax - cy*ay, so pre = xs[j]*ax + ys[i]*ay + c.
  - create: PE matmul K=3: lhsT = Wpre[3, 128(n,k)] per block,
    rhs = G[3, 512 pixels] (xs/ys/1 rows) -> psum pre[128, 512].
  - relu: ACT/DVE psum->SBUF.
  - contract: PE matmul K=128 with zero-padded block-diagonal
    lhsT[128(n,k), 128 n-cols]; 16 blocks accumulate into ONE psum bank
    -> dense [128 n, 512 pix] output; + b2 via per-partition ts-add.
All matmuls in float32r (full-speed fp32 path, ~3e-4 rel err).
"""
import numpy as np
from contextlib import ExitStack

# ---------------- problem constants (hardcoded per spec) ----------------
B_, Q_, DIM = 8, 300, 256
HD = 16            # hidden_dim
H = W = 64
NPIX = H * W       # 4096
CH = 512           # pixels per chunk
NCH = NPIX // CH   # 8
NCORE = 8
NPER = (B_ * Q_) // NCORE     # 300 queries per core
NPAD = 304                    # padded to 38 blocks of 8
NBLK = NPAD // 8              # 38
# psum groups: 16 blocks = 128 n rows per group (last group 6 blocks / 48 rows)
GROUPS = [(0, 16, 128), (16, 16, 128), (32, 6, 48)]
CGRP = 4                      # chunks per chunk-group (psum out banks)

_CACHE = {}


def _install_patches():
    """Shims for this container: walrus here rejects >1 sem-wait per
    instruction; split extra waits onto same-engine NOP carriers."""
    import bass_rust
    import concourse.tile as tile

    def _drain_and_barrier(self, tick_clock, wait_clock):
        nc = self.nc
        probe = nc.sync.nop(nofuse=True, hint="tile_drain_probe")
        wait_clock.add_sem_waits(
            probe.ins, bass_rust.ScopedClock({None: tick_clock.global_clock})
        )
        waits = list(probe.ins.sync_info.on_wait)
        probe.ins.sync_info.on_wait = []
        for i, w in enumerate(waits):
            carrier = nc.sync.nop(nofuse=True, hint=f"tile_drain_wait{i}")
            si = carrier.ins.sync_info
            if si is None:
                si = bass_rust.SyncInfo(on_wait=[], on_update=[])
                carrier.ins.sync_info = si
            si.on_wait = [w]
        nc.sync.drain()
        nc.all_engine_barrier()
        assert self.sems is not None
        popped = nc._tile_sem_poison_stack.pop()
        assert popped is self._sem_poison
        nc.clear_and_free_semaphores(list(self.sems.allocated().values()))
        nc.all_engine_barrier()

    tile.TileContext._drain_and_barrier = _drain_and_barrier


def _split_waits(nc, max_waits=1):
    import concourse.mybir as mybir

    for f in nc.m.functions:
        for bb in f.blocks:
            insts = bb.instructions
            out = []
            changed = False
            for inst in insts:
                si = inst.sync_info
                waits = list(si.on_wait) if si is not None else []
                if len(waits) > max_waits:
                    keep = waits[:max_waits]
                    rest = waits[max_waits:]
                    for j in range(0, len(rest), max_waits):
                        out.append(
                            mybir.InstNoOp(
                                name=f"{inst.name}_wsplit{j}",
                                engine=inst.engine,
                                bass_nofuse=True,
                                sync_info=mybir.SyncInfo(
                                    on_wait=rest[j : j + max_waits], on_update=[]
                                ),
                            )
                        )
                    si.on_wait = keep
                    changed = True
                out.append(inst)
            if changed:
                insts[:] = out


PREC = "bf16"   # "bf16" or "f32r" for the create/contract matmuls


def _build(prec=None):
    import concourse.bass as bass
    import concourse.tile as tile
    import concourse.mybir as mybir
    from concourse.masks import make_identity

    if prec is None:
        prec = PREC
    _install_patches()
    f32 = mybir.dt.float32
    f32r = mybir.dt.float32r
    mmdt = mybir.dt.bfloat16 if prec == "bf16" else f32r
    AO = mybir.AluOpType
    Relu = mybir.ActivationFunctionType.Relu

    nc = bass.Bass()
    qT_t = nc.dram_tensor("qT", (DIM, NPAD), f32, kind="ExternalInput")
    pos_t = nc.dram_tensor("posn", (NPAD, 4), f32, kind="ExternalInput")
    wg_t = nc.dram_tensor("wg", (DIM, 4 * HD + 1), f32, kind="ExternalInput")
    bg_t = nc.dram_tensor("bgr", (1, 4 * HD + 1), f32, kind="ExternalInput")
    g_t = nc.dram_tensor("gx", (3, NPIX), mmdt, kind="ExternalInput")
    out_t = nc.dram_tensor("out", (NPAD, NPIX), f32, kind="ExternalOutput")

    NC65 = 4 * HD + 1

    with ExitStack() as ctx:
        tc = ctx.enter_context(tile.TileContext(nc))
        sb = ctx.enter_context(tc.tile_pool(name="sb", bufs=1))

        # ---------------- load inputs ----------------
        qt0 = sb.tile([128, NPAD], f32, tag="qt0")
        nc.sync.dma_start(qt0[:], qT_t[0:128, :])
        qt1 = sb.tile([128, NPAD], f32, tag="qt1")
        nc.sync.dma_start(qt1[:], qT_t[128:256, :])
        wg0 = sb.tile([128, NC65], f32, tag="wg0")
        nc.sync.dma_start(wg0[:], wg_t[0:128, :])
        wg1 = sb.tile([128, NC65], f32, tag="wg1")
        nc.sync.dma_start(wg1[:], wg_t[128:256, :])
        bgr = sb.tile([1, NC65], f32, tag="bgr")
        nc.sync.dma_start(bgr[:], bg_t[:])
        posn = sb.tile([128, 12], f32, tag="posn")
        g_sb = sb.tile([3, NPIX], mmdt, tag="gsb")
        nc.sync.dma_start(g_sb[:], g_t[:])

        ones1 = sb.tile([1, 128], f32, tag="ones1")
        nc.vector.memset(ones1[:], 1.0)
        ident = sb.tile([128, 128], f32, tag="ident")
        make_identity(nc, ident)

        inv = sb.tile([128, 6], f32, tag="inv")
        wn = sb.tile([128, 3 * NC65], f32, tag="wn")
        axn = sb.tile([128, 48], f32, tag="axn")
        ayn = sb.tile([128, 48], f32, tag="ayn")
        cn = sb.tile([128, 48], f32, tag="cn")
        tmp = sb.tile([128, 48], f32, tag="tmp")
        b2n = sb.tile([128, 3], f32, tag="b2n")
        w2t = sb.tile([HD, NPAD], mmdt, tag="w2t")
        axm = sb.tile([128, 48], mmdt, tag="axm")
        aym = sb.tile([128, 48], mmdt, tag="aym")
        cm = sb.tile([128, 48], mmdt, tag="cm")
        wpre = sb.tile([3, NPAD * HD], mmdt, tag="wpre")
        w2big = sb.tile([128, NBLK * 128], mmdt, tag="w2big")
        if prec == "bf16":
            nc.gpsimd.memset(w2big[:], 0.0)
        else:
            nc.gpsimd.memset(w2big[:].bitcast(f32), 0.0)

        with tc.tile_pool(name="setps", bufs=2, space="PSUM") as setps:
            for ci, (r0, nr) in enumerate([(0, 128), (128, 128), (256, 48)]):
                nc.sync.dma_start(posn[0:nr, 4 * ci:4 * ci + 4], pos_t[r0:r0 + nr, :])
                nc.vector.reciprocal(inv[0:nr, 2 * ci:2 * ci + 2],
                                     posn[0:nr, 4 * ci + 2:4 * ci + 4])
                # weights by-n = qT_chunk.T @ Wg + bg
                wps = setps.tile([128, NC65], f32, tag="wps")
                nc.tensor.matmul(wps[0:nr, :], qt0[:, r0:r0 + nr], wg0[:],
                                 start=True, stop=False)
                nc.tensor.matmul(wps[0:nr, :], qt1[:, r0:r0 + nr], wg1[:],
                                 start=False, stop=False)
                nc.tensor.matmul(wps[0:nr, :], ones1[:, 0:nr], bgr[:],
                                 start=False, stop=True)
                wnc = wn[0:nr, NC65 * ci:NC65 * ci + NC65]
                nc.vector.tensor_copy(wnc, wps[0:nr, :])

                axc = axn[0:nr, 16 * ci:16 * ci + 16]
                ayc = ayn[0:nr, 16 * ci:16 * ci + 16]
                cc = cn[0:nr, 16 * ci:16 * ci + 16]
                tc_ = tmp[0:nr, 16 * ci:16 * ci + 16]
                ibw = inv[0:nr, 2 * ci:2 * ci + 1]
                ibh = inv[0:nr, 2 * ci + 1:2 * ci + 2]
                cx = posn[0:nr, 4 * ci:4 * ci + 1]
                cy = posn[0:nr, 4 * ci + 1:4 * ci + 2]
                nc.vector.tensor_scalar(axc, wnc[:, 0:16], ibw, None, AO.mult)
                nc.vector.tensor_scalar(ayc, wnc[:, 16:32], ibh, None, AO.mult)
                nc.vector.tensor_scalar(tc_, axc, cx, None, AO.mult)
                nc.vector.tensor_sub(cc, wnc[:, 32:48], tc_)
                nc.vector.tensor_scalar(tc_, ayc, cy, None, AO.mult)
                nc.vector.tensor_sub(cc, cc, tc_)
                axmc = axm[0:nr, 16 * ci:16 * ci + 16]
                aymc = aym[0:nr, 16 * ci:16 * ci + 16]
                cmc = cm[0:nr, 16 * ci:16 * ci + 16]
                nc.vector.tensor_copy(axmc, axc)
                nc.vector.tensor_copy(aymc, ayc)
                nc.vector.tensor_copy(cmc, cc)
                nc.scalar.copy(b2n[0:nr, ci:ci + 1], wnc[:, 64:65])

                # w2 (by-n) -> transposed [16, n]
                wtp = setps.tile([HD, 128], f32, tag="wtp")
                nc.tensor.transpose(wtp[:, 0:nr], wnc[:, 48:64], ident[0:nr, 0:nr])
                nc.vector.tensor_copy(w2t[:, r0:r0 + nr], wtp[:, 0:nr])

                # Wpre rows: partition->free flatten  [nr, 16] -> [1, nr*16]
                for r, src_ap in ((0, axmc), (1, aymc), (2, cmc)):
                    dst = wpre[r:r + 1, HD * r0: HD * r0 + HD * nr]
                    nc.sync.dma_start(
                        dst.rearrange("p (n k) -> p n k", k=HD), src_ap)

            # w2big: zero-padded block-diagonal lhsT per block
            # col(B, m) = 128*B + 8*(B%16) + m ; partition rows 16m..16m+16
            # B = 16g + b -> col = 2048*g + 136*b + m
            for m in range(8):
                t = w2big[16 * m:16 * m + 16, 0:1].tensor
                ts_ = w2t[:, 0:1].tensor
                for g0, nb in ((0, 16), (1, 16), (2, 6)):
                    dst = bass.AP(t, (16 * m) * (NBLK * 128) + 2048 * g0 + m,
                                  [[NBLK * 128, 16], [136, nb]])
                    src = bass.AP(ts_, 128 * g0 + m,
                                  [[NPAD, 16], [8, nb]])
                    nc.sync.dma_start(dst, src)

        # ---------------- main loop ----------------
        pre_pool = ctx.enter_context(tc.tile_pool(name="prep", bufs=3, space="PSUM"))
        out_pool = ctx.enter_context(tc.tile_pool(name="outp", bufs=4, space="PSUM"))
        act_pool = ctx.enter_context(tc.tile_pool(name="actp", bufs=6))
        osb_pool = ctx.enter_context(tc.tile_pool(name="osbp", bufs=3))

        rl = 0  # relu parity for ACT/DVE balance
        for (b0, nblk, nrows) in GROUPS:
            gbase = 8 * b0
            for cg in range(NCH // CGRP):
                outs = [out_pool.tile([128, CH], f32, tag="ops", name=f"ops{cg}_{i}")
                        for i in range(CGRP)]
                for b in range(nblk):
                    Bb = b0 + b
                    acts = []
                    for ci in range(CGRP):
                        c = CGRP * cg + ci
                        pre = pre_pool.tile([128, CH], f32, tag="pre")
                        nc.tensor.matmul(pre[:],
                                         wpre[:, 128 * Bb:128 * Bb + 128],
                                         g_sb[:, CH * c:CH * c + CH],
                                         start=True, stop=True)
                        act = act_pool.tile([128, CH], mmdt, tag="act")
                        if rl % 2 == 0:
                            nc.scalar.activation(act[:], pre[:], Relu)
                        else:
                            nc.vector.tensor_scalar(act[:], pre[:], 0.0, None, AO.max)
                        rl += 1
                        acts.append(act)
                    for ci in range(CGRP):
                        nc.tensor.matmul(outs[ci][0:nrows, :],
                                         w2big[:, 128 * Bb:128 * Bb + 128][:, 0:nrows],
                                         acts[ci][:],
                                         start=(b == 0), stop=(b == nblk - 1),
                                         skip_group_check=True)
                for ci in range(CGRP):
                    c = CGRP * cg + ci
                    osb = osb_pool.tile([128, CH], f32, tag="osb")
                    gi = b0 // 16
                    nc.scalar.add(osb[0:nrows, :], outs[ci][0:nrows, :],
                                  b2n[0:nrows, gi:gi + 1])
                    nc.sync.dma_start(out_t[gbase:gbase + nrows, CH * c:CH * c + CH],
                                      osb[0:nrows, :])

    _split_waits(nc)
    return nc


def _get_nc():
    if "nc" not in _CACHE:
        _CACHE["nc"] = _build()
    return _CACHE["nc"]


def _prepare_in_maps(pos, queries, Wg, bg):
    pos_f = np.asarray(pos, np.float32).reshape(-1, 4)
    q_f = np.asarray(queries, np.float32).reshape(-1, DIM)
    Wg = np.ascontiguousarray(np.asarray(Wg, np.float32))
    bg = np.ascontiguousarray(np.asarray(bg, np.float32).reshape(1, 4 * HD + 1))

    xs = (np.arange(W, dtype=np.float32) + 0.5) / W
    ys = (np.arange(H, dtype=np.float32) + 0.5) / H
    gx = np.ascontiguousarray(
        np.stack([np.tile(xs, H), np.repeat(ys, W),
                  np.ones(NPIX, np.float32)]).astype(np.float32))
    if PREC == "bf16":
        import ml_dtypes
        gx = gx.astype(ml_dtypes.bfloat16)

    in_maps = []
    for ci in range(NCORE):
        rows = slice(NPER * ci, NPER * (ci + 1))
        qT = np.zeros((DIM, NPAD), np.float32)
        qT[:, :NPER] = q_f[rows].T
        posn = np.tile(np.array([0.5, 0.5, 1.0, 1.0], np.float32), (NPAD, 1))
        posn[:NPER] = pos_f[rows]
        in_maps.append({"qT": qT, "posn": posn, "wg": Wg, "bgr": bg, "gx": gx})
    return in_maps


def kernel(pos, queries, Wg, bg, size):
    from concourse.bass_utils import run_bass_kernel_spmd

    assert int(size) == 64
    in_maps = _prepare_in_maps(pos, queries, Wg, bg)
    res = run_bass_kernel_spmd(_get_nc(), in_maps, list(range(NCORE)), trace=False)
    outs = [res.results[i]["out"][:NPER].reshape(NPER, H, W) for i in range(NCORE)]
    return np.concatenate(outs, 0).reshape(B_, Q_, H, W).astype(np.float32)


# revision 9
# speedup vs baseline: 2.1953x; 1.8072x over previous
"""Trainium2 Bass kernel for nn_PosMLP (box-relative position MLP).

Math (per query n):
  weights = q @ Wg + bg                       [N, 65]
  w1x = weights[:, 0:16], w1y = weights[:, 16:32]
  b1  = weights[:, 32:48], w2  = weights[:, 48:64], b2 = weights[:, 64]
  rel_x[j] = (xs[j] - cx)/bw ; rel_y[i] = (ys[i] - cy)/bh
  pre[i,j,k] = rel_x[j]*w1x[k] + rel_y[i]*w1y[k] + b1[k]
  out[i,j] = sum_k relu(pre)[i,j,k] * w2[k] + b2           (64x64 image)

Device mapping (per core, N=300 queries padded to 304 = 38 blocks of 8):
  - fold pos into per-(n,k) affine coeffs: ax = w1x/bw, ay = w1y/bh,
    c = b1 - cx*ax - cy*ay, so pre = xs[j]*ax + ys[i]*ay + c.
  - create: PE matmul K=3: lhsT = Wpre[3, 128(n,k)] per block,
    rhs = G[3, 512 pixels] (xs/ys/1 rows) -> psum pre[128, 512].
  - relu: ACT/DVE psum->SBUF.
  - contract: PE matmul K=128 with zero-padded block-diagonal
    lhsT[128(n,k), 128 n-cols]; 16 blocks accumulate into ONE psum bank
    -> dense [128 n, 512 pix] output; + b2 via per-partition ts-add.
All matmuls in float32r (full-speed fp32 path, ~3e-4 rel err).
"""
import numpy as np
from contextlib import ExitStack

# ---------------- problem constants (hardcoded per spec) ----------------
B_, Q_, DIM = 8, 300, 256
HD = 16            # hidden_dim
H = W = 64
NPIX = H * W       # 4096
CH = 512           # pixels per chunk
NCH = NPIX // CH   # 8
NCORE = 8
NPER = (B_ * Q_) // NCORE     # 300 queries per core
NPAD = 304                    # padded to 38 blocks of 8
NBLK = NPAD // 8              # 38
# psum groups: 16 blocks = 128 n rows per group (last group 6 blocks / 48 rows)
GROUPS = [(0, 16, 128), (16, 16, 128), (32, 6, 48)]
CGRP = 4                      # chunks per chunk-group (psum out banks)

_CACHE = {}


def _install_patches():
    """Shims for this container: walrus here rejects >1 sem-wait per
    instruction; split extra waits onto same-engine NOP carriers."""
    import bass_rust
    import concourse.tile as tile

    def _drain_and_barrier(self, tick_clock, wait_clock):
        nc = self.nc
        probe = nc.sync.nop(nofuse=True, hint="tile_drain_probe")
        wait_clock.add_sem_waits(
            probe.ins, bass_rust.ScopedClock({None: tick_clock.global_clock})
        )
        waits = list(probe.ins.sync_info.on_wait)
        probe.ins.sync_info.on_wait = []
        for i, w in enumerate(waits):
            carrier = nc.sync.nop(nofuse=True, hint=f"tile_drain_wait{i}")
            si = carrier.ins.sync_info
            if si is None:
                si = bass_rust.SyncInfo(on_wait=[], on_update=[])
                carrier.ins.sync_info = si
            si.on_wait = [w]
        nc.sync.drain()
        nc.all_engine_barrier()
        assert self.sems is not None
        popped = nc._tile_sem_poison_stack.pop()
        assert popped is self._sem_poison
        nc.clear_and_free_semaphores(list(self.sems.allocated().values()))
        nc.all_engine_barrier()

    tile.TileContext._drain_and_barrier = _drain_and_barrier


def _split_waits(nc, max_waits=1):
    import concourse.mybir as mybir

    for f in nc.m.functions:
        for bb in f.blocks:
            insts = bb.instructions
            out = []
            changed = False
            for inst in insts:
                si = inst.sync_info
                waits = list(si.on_wait) if si is not None else []
                if len(waits) > max_waits:
                    keep = waits[:max_waits]
                    rest = waits[max_waits:]
                    for j in range(0, len(rest), max_waits):
                        out.append(
                            mybir.InstNoOp(
                                name=f"{inst.name}_wsplit{j}",
                                engine=inst.engine,
                                bass_nofuse=True,
                                sync_info=mybir.SyncInfo(
                                    on_wait=rest[j : j + max_waits], on_update=[]
                                ),
                            )
                        )
                    si.on_wait = keep
                    changed = True
                out.append(inst)
            if changed:
                insts[:] = out


PREC = "bf16"   # "bf16" or "f32r" for the create/contract matmuls


def _build(prec=None):
    import concourse.bass as bass
    import concourse.tile as tile
    import concourse.mybir as mybir
    from concourse.masks import make_identity

    if prec is None:
        prec = PREC
    _install_patches()
    f32 = mybir.dt.float32
    f32r = mybir.dt.float32r
    mmdt = mybir.dt.bfloat16 if prec == "bf16" else f32r
    AO = mybir.AluOpType
    Relu = mybir.ActivationFunctionType.Relu

    nc = bass.Bass()
    qT_t = nc.dram_tensor("qT", (DIM, NPAD), f32, kind="ExternalInput")
    pos_t = nc.dram_tensor("posn", (NPAD, 4), f32, kind="ExternalInput")
    wg_t = nc.dram_tensor("wg", (DIM, 4 * HD + 1), f32, kind="ExternalInput")
    bg_t = nc.dram_tensor("bgr", (1, 4 * HD + 1), f32, kind="ExternalInput")
    g_t = nc.dram_tensor("gx", (99, NPIX), mmdt, kind="ExternalInput")
    out_t = nc.dram_tensor("out", (NPAD, NPIX), f32, kind="ExternalOutput")

    NC65 = 4 * HD + 1

    with ExitStack() as ctx:
        tc = ctx.enter_context(tile.TileContext(nc))
        sb = ctx.enter_context(tc.tile_pool(name="sb", bufs=1))

        # ---------------- load inputs ----------------
        qt0 = sb.tile([128, NPAD], f32, tag="qt0")
        nc.sync.dma_start(qt0[:], qT_t[0:128, :])
        qt1 = sb.tile([128, NPAD], f32, tag="qt1")
        nc.sync.dma_start(qt1[:], qT_t[128:256, :])
        wg0 = sb.tile([128, NC65], f32, tag="wg0")
        nc.sync.dma_start(wg0[:], wg_t[0:128, :])
        wg1 = sb.tile([128, NC65], f32, tag="wg1")
        nc.sync.dma_start(wg1[:], wg_t[128:256, :])
        bgr = sb.tile([1, NC65], f32, tag="bgr")
        nc.sync.dma_start(bgr[:], bg_t[:])
        posn = sb.tile([128, 12], f32, tag="posn")
        g_sb = sb.tile([99, NPIX], mmdt, tag="gsb")
        nc.sync.dma_start(g_sb[:], g_t[:])

        ones1 = sb.tile([1, 128], f32, tag="ones1")
        nc.vector.memset(ones1[:], 1.0)
        ident = sb.tile([128, 128], f32, tag="ident")
        make_identity(nc, ident)

        inv = sb.tile([128, 6], f32, tag="inv")
        wn = sb.tile([128, 3 * NC65], f32, tag="wn")
        axn = sb.tile([128, 48], f32, tag="axn")
        ayn = sb.tile([128, 48], f32, tag="ayn")
        cn = sb.tile([128, 48], f32, tag="cn")
        tmp = sb.tile([128, 48], f32, tag="tmp")
        b2n = sb.tile([128, 3], f32, tag="b2n")
        w2t = sb.tile([HD, NPAD], mmdt, tag="w2t")
        axm = sb.tile([128, 48], mmdt, tag="axm")
        aym = sb.tile([128, 48], mmdt, tag="aym")
        cm = sb.tile([128, 48], mmdt, tag="cm")
        wpre = sb.tile([99, NPAD * HD], mmdt, tag="wpre")
        w2big = sb.tile([128, NBLK * 128], mmdt, tag="w2big")
        if prec == "bf16":
            nc.gpsimd.memset(w2big[:], 0.0)
        else:
            nc.gpsimd.memset(w2big[:].bitcast(f32), 0.0)

        with tc.tile_pool(name="setps", bufs=2, space="PSUM") as setps:
            for ci, (r0, nr) in enumerate([(0, 128), (128, 128), (256, 48)]):
                nc.sync.dma_start(posn[0:nr, 4 * ci:4 * ci + 4], pos_t[r0:r0 + nr, :])
                nc.vector.reciprocal(inv[0:nr, 2 * ci:2 * ci + 2],
                                     posn[0:nr, 4 * ci + 2:4 * ci + 4])
                # weights by-n = qT_chunk.T @ Wg + bg
                wps = setps.tile([128, NC65], f32, tag="wps")
                nc.tensor.matmul(wps[0:nr, :], qt0[:, r0:r0 + nr], wg0[:],
                                 start=True, stop=False)
                nc.tensor.matmul(wps[0:nr, :], qt1[:, r0:r0 + nr], wg1[:],
                                 start=False, stop=False)
                nc.tensor.matmul(wps[0:nr, :], ones1[:, 0:nr], bgr[:],
                                 start=False, stop=True)
                wnc = wn[0:nr, NC65 * ci:NC65 * ci + NC65]
                nc.vector.tensor_copy(wnc, wps[0:nr, :])

                axc = axn[0:nr, 16 * ci:16 * ci + 16]
                ayc = ayn[0:nr, 16 * ci:16 * ci + 16]
                cc = cn[0:nr, 16 * ci:16 * ci + 16]
                tc_ = tmp[0:nr, 16 * ci:16 * ci + 16]
                ibw = inv[0:nr, 2 * ci:2 * ci + 1]
                ibh = inv[0:nr, 2 * ci + 1:2 * ci + 2]
                cx = posn[0:nr, 4 * ci:4 * ci + 1]
                cy = posn[0:nr, 4 * ci + 1:4 * ci + 2]
                nc.vector.tensor_scalar(axc, wnc[:, 0:16], ibw, None, AO.mult)
                nc.vector.tensor_scalar(ayc, wnc[:, 16:32], ibh, None, AO.mult)
                nc.vector.tensor_scalar(tc_, axc, cx, None, AO.mult)
                nc.vector.tensor_sub(cc, wnc[:, 32:48], tc_)
                nc.vector.tensor_scalar(tc_, ayc, cy, None, AO.mult)
                nc.vector.tensor_sub(cc, cc, tc_)
                axmc = axm[0:nr, 16 * ci:16 * ci + 16]
                aymc = aym[0:nr, 16 * ci:16 * ci + 16]
                cmc = cm[0:nr, 16 * ci:16 * ci + 16]
                nc.vector.tensor_copy(axmc, axc)
                nc.vector.tensor_copy(aymc, ayc)
                nc.vector.tensor_copy(cmc, cc)
                nc.scalar.copy(b2n[0:nr, ci:ci + 1], wnc[:, 64:65])

                # w2 (by-n) -> transposed [16, n]
                wtp = setps.tile([HD, 128], f32, tag="wtp")
                nc.tensor.transpose(wtp[:, 0:nr], wnc[:, 48:64], ident[0:nr, 0:nr])
                nc.vector.tensor_copy(w2t[:, r0:r0 + nr], wtp[:, 0:nr])

                # Wpre rows: partition->free flatten  [nr, 16] -> [1, nr*16]
                for r, src_ap in ((0, axmc), (1, aymc), (2, cmc)):
                    dst = wpre[r:r + 1, HD * r0: HD * r0 + HD * nr]
                    nc.sync.dma_start(
                        dst.rearrange("p (n k) -> p n k", k=HD), src_ap)

            for s in range(1, 4):
                nc.sync.dma_start(wpre[32 * s:32 * s + 3, :], wpre[0:3, :])

            # w2big: zero-padded block-diagonal lhsT per block
            # col(B, m) = 128*B + 8*(B%16) + m ; partition rows 16m..16m+16
            # B = 16g + b -> col = 2048*g + 136*b + m
            for m in range(8):
                t = w2big[16 * m:16 * m + 16, 0:1].tensor
                ts_ = w2t[:, 0:1].tensor
                for g0, nb in ((0, 16), (1, 16), (2, 6)):
                    dst = bass.AP(t, (16 * m) * (NBLK * 128) + 2048 * g0 + m,
                                  [[NBLK * 128, 16], [136, nb]])
                    src = bass.AP(ts_, 128 * g0 + m,
                                  [[NPAD, 16], [8, nb]])
                    nc.sync.dma_start(dst, src)

        # ---------------- main loop ----------------
        pre_pool = ctx.enter_context(tc.tile_pool(name="prep", bufs=4, space="PSUM"))
        out_pool = ctx.enter_context(tc.tile_pool(name="outp", bufs=4, space="PSUM"))
        act_pool = ctx.enter_context(tc.tile_pool(name="actp", bufs=10))
        osb_pool = ctx.enter_context(tc.tile_pool(name="osbp", bufs=3))

        rl = 0  # relu parity for ACT/DVE balance
        for (b0, nblk, nrows) in GROUPS:
            gbase = 8 * b0
            for cg in range(NCH // CGRP):
                outs = [out_pool.tile([128, CH], f32, tag="ops", name=f"ops{cg}_{i}")
                        for i in range(CGRP)]
                pending = None  # (b, acts) awaiting contract, one block behind

                def flush(pending):
                    b, acts = pending
                    Bb = b0 + b
                    for ci in range(CGRP):
                        nc.tensor.matmul(outs[ci][0:nrows, :],
                                         w2big[:, 128 * Bb:128 * Bb + 128][:, 0:nrows],
                                         acts[ci][:],
                                         start=(b == 0), stop=(b == nblk - 1),
                                         skip_group_check=True)

                for b in range(nblk):
                    Bb = b0 + b
                    acts = []
                    for ci in range(CGRP):
                        c = CGRP * cg + ci
                        pre = pre_pool.tile([128, CH], f32, tag="pre")
                        nc.tensor.matmul(pre[:],
                                         wpre[32 * ci:32 * ci + 3,
                                              128 * Bb:128 * Bb + 128],
                                         g_sb[32 * ci:32 * ci + 3,
                                              CH * c:CH * c + CH],
                                         start=True, stop=True,
                                         tile_position=(32 * ci, 0))
                        act = act_pool.tile([128, CH], mmdt, tag="act")
                        if rl % 2 == 0:
                            nc.scalar.activation(act[:], pre[:], Relu)
                        else:
                            nc.vector.tensor_scalar(act[:], pre[:], 0.0, None, AO.max)
                        rl += 1
                        acts.append(act)
                    if pending is not None:
                        flush(pending)
                    pending = (b, acts)
                flush(pending)
                for ci in range(CGRP):
                    c = CGRP * cg + ci
                    osb = osb_pool.tile([128, CH], f32, tag="osb")
                    gi = b0 // 16
                    nc.scalar.add(osb[0:nrows, :], outs[ci][0:nrows, :],
                                  b2n[0:nrows, gi:gi + 1])
                    nc.sync.dma_start(out_t[gbase:gbase + nrows, CH * c:CH * c + CH],
                                      osb[0:nrows, :])

    _split_waits(nc)
    return nc


def _get_nc():
    if "nc" not in _CACHE:
        _CACHE["nc"] = _build()
    return _CACHE["nc"]


def _prepare_in_maps(pos, queries, Wg, bg):
    pos_f = np.asarray(pos, np.float32).reshape(-1, 4)
    q_f = np.asarray(queries, np.float32).reshape(-1, DIM)
    Wg = np.ascontiguousarray(np.asarray(Wg, np.float32))
    bg = np.ascontiguousarray(np.asarray(bg, np.float32).reshape(1, 4 * HD + 1))

    xs = (np.arange(W, dtype=np.float32) + 0.5) / W
    ys = (np.arange(H, dtype=np.float32) + 0.5) / H
    g3 = np.stack([np.tile(xs, H), np.repeat(ys, W),
                   np.ones(NPIX, np.float32)]).astype(np.float32)
    gx = np.zeros((99, NPIX), np.float32)
    for r in range(4):
        gx[32 * r:32 * r + 3] = g3
    if PREC == "bf16":
        import ml_dtypes
        gx = gx.astype(ml_dtypes.bfloat16)
    gx = np.ascontiguousarray(gx)

    in_maps = []
    for ci in range(NCORE):
        rows = slice(NPER * ci, NPER * (ci + 1))
        qT = np.zeros((DIM, NPAD), np.float32)
        qT[:, :NPER] = q_f[rows].T
        posn = np.tile(np.array([0.5, 0.5, 1.0, 1.0], np.float32), (NPAD, 1))
        posn[:NPER] = pos_f[rows]
        in_maps.append({"qT": qT, "posn": posn, "wg": Wg, "bgr": bg, "gx": gx})
    return in_maps


def kernel(pos, queries, Wg, bg, size):
    from concourse.bass_utils import run_bass_kernel_spmd

    assert int(size) == 64
    in_maps = _prepare_in_maps(pos, queries, Wg, bg)
    res = run_bass_kernel_spmd(_get_nc(), in_maps, list(range(NCORE)), trace=False)
    outs = [res.results[i]["out"][:NPER].reshape(NPER, H, W) for i in range(NCORE)]
    return np.concatenate(outs, 0).reshape(B_, Q_, H, W).astype(np.float32)


# revision 11
# speedup vs baseline: 2.2096x; 1.0065x over previous
"""Trainium2 Bass kernel for nn_PosMLP (box-relative position MLP).

Math (per query n):
  weights = q @ Wg + bg                       [N, 65]
  w1x = weights[:, 0:16], w1y = weights[:, 16:32]
  b1  = weights[:, 32:48], w2  = weights[:, 48:64], b2 = weights[:, 64]
  rel_x[j] = (xs[j] - cx)/bw ; rel_y[i] = (ys[i] - cy)/bh
  pre[i,j,k] = rel_x[j]*w1x[k] + rel_y[i]*w1y[k] + b1[k]
  out[i,j] = sum_k relu(pre)[i,j,k] * w2[k] + b2           (64x64 image)

Device mapping (per core, N=300 queries padded to 304 = 38 blocks of 8):
  - fold pos into per-(n,k) affine coeffs: ax = w1x/bw, ay = w1y/bh,
    c = b1 - cx*ax - cy*ay, so pre = xs[j]*ax + ys[i]*ay + c.
  - create: PE matmul K=3: lhsT = Wpre[3, 128(n,k)] per block,
    rhs = G[3, 512 pixels] (xs/ys/1 rows) -> psum pre[128, 512].
  - relu: ACT/DVE psum->SBUF.
  - contract: PE matmul K=128 with zero-padded block-diagonal
    lhsT[128(n,k), 128 n-cols]; 16 blocks accumulate into ONE psum bank
    -> dense [128 n, 512 pix] output; + b2 via per-partition ts-add.
All matmuls in float32r (full-speed fp32 path, ~3e-4 rel err).
"""
import numpy as np
from contextlib import ExitStack

# ---------------- problem constants (hardcoded per spec) ----------------
B_, Q_, DIM = 8, 300, 256
HD = 16            # hidden_dim
H = W = 64
NPIX = H * W       # 4096
CH = 512           # pixels per chunk
NCH = NPIX // CH   # 8
NCORE = 8
NPER = (B_ * Q_) // NCORE     # 300 queries per core
NPAD = 304                    # padded to 38 blocks of 8
NBLK = NPAD // 8              # 38
# psum groups: 16 blocks = 128 n rows per group (last group 6 blocks / 48 rows)
GROUPS = [(0, 16, 128), (16, 16, 128), (32, 6, 48)]
CGRP = 4                      # chunks per chunk-group (psum out banks)

_CACHE = {}


def _install_patches():
    """Shims for this container: walrus here rejects >1 sem-wait per
    instruction; split extra waits onto same-engine NOP carriers."""
    import bass_rust
    import concourse.tile as tile

    def _drain_and_barrier(self, tick_clock, wait_clock):
        nc = self.nc
        probe = nc.sync.nop(nofuse=True, hint="tile_drain_probe")
        wait_clock.add_sem_waits(
            probe.ins, bass_rust.ScopedClock({None: tick_clock.global_clock})
        )
        waits = list(probe.ins.sync_info.on_wait)
        probe.ins.sync_info.on_wait = []
        for i, w in enumerate(waits):
            carrier = nc.sync.nop(nofuse=True, hint=f"tile_drain_wait{i}")
            si = carrier.ins.sync_info
            if si is None:
                si = bass_rust.SyncInfo(on_wait=[], on_update=[])
                carrier.ins.sync_info = si
            si.on_wait = [w]
        nc.sync.drain()
        nc.all_engine_barrier()
        assert self.sems is not None
        popped = nc._tile_sem_poison_stack.pop()
        assert popped is self._sem_poison
        nc.clear_and_free_semaphores(list(self.sems.allocated().values()))
        nc.all_engine_barrier()

    tile.TileContext._drain_and_barrier = _drain_and_barrier


def _split_waits(nc, max_waits=1):
    import concourse.mybir as mybir

    for f in nc.m.functions:
        for bb in f.blocks:
            insts = bb.instructions
            out = []
            changed = False
            for inst in insts:
                si = inst.sync_info
                waits = list(si.on_wait) if si is not None else []
                if len(waits) > max_waits:
                    keep = waits[:max_waits]
                    rest = waits[max_waits:]
                    for j in range(0, len(rest), max_waits):
                        out.append(
                            mybir.InstNoOp(
                                name=f"{inst.name}_wsplit{j}",
                                engine=inst.engine,
                                bass_nofuse=True,
                                sync_info=mybir.SyncInfo(
                                    on_wait=rest[j : j + max_waits], on_update=[]
                                ),
                            )
                        )
                    si.on_wait = keep
                    changed = True
                out.append(inst)
            if changed:
                insts[:] = out


PREC = "bf16"   # "bf16" or "f32r" for the create/contract matmuls


def _build(prec=None):
    import concourse.bass as bass
    import concourse.tile as tile
    import concourse.mybir as mybir
    from concourse.masks import make_identity

    if prec is None:
        prec = PREC
    _install_patches()
    f32 = mybir.dt.float32
    f32r = mybir.dt.float32r
    mmdt = mybir.dt.bfloat16 if prec == "bf16" else f32r
    AO = mybir.AluOpType
    Relu = mybir.ActivationFunctionType.Relu

    nc = bass.Bass()
    qT_t = nc.dram_tensor("qT", (DIM, NPAD), f32, kind="ExternalInput")
    pos_t = nc.dram_tensor("posn", (NPAD, 4), f32, kind="ExternalInput")
    wg_t = nc.dram_tensor("wg", (DIM, 4 * HD + 1), f32, kind="ExternalInput")
    bg_t = nc.dram_tensor("bgr", (1, 4 * HD + 1), f32, kind="ExternalInput")
    g_t = nc.dram_tensor("gx", (99, NPIX), mmdt, kind="ExternalInput")
    out_t = nc.dram_tensor("out", (NPAD, NPIX), f32, kind="ExternalOutput")

    NC65 = 4 * HD + 1

    with ExitStack() as ctx:
        tc = ctx.enter_context(tile.TileContext(nc))
        sb = ctx.enter_context(tc.tile_pool(name="sb", bufs=1))

        # ---------------- load inputs ----------------
        qt0 = sb.tile([128, NPAD], f32, tag="qt0")
        nc.sync.dma_start(qt0[:], qT_t[0:128, :])
        qt1 = sb.tile([128, NPAD], f32, tag="qt1")
        nc.sync.dma_start(qt1[:], qT_t[128:256, :])
        wg0 = sb.tile([128, NC65], f32, tag="wg0")
        nc.sync.dma_start(wg0[:], wg_t[0:128, :])
        wg1 = sb.tile([128, NC65], f32, tag="wg1")
        nc.sync.dma_start(wg1[:], wg_t[128:256, :])
        bgr = sb.tile([1, NC65], f32, tag="bgr")
        nc.sync.dma_start(bgr[:], bg_t[:])
        posn = sb.tile([128, 12], f32, tag="posn")
        g_sb = sb.tile([99, NPIX], mmdt, tag="gsb")
        nc.sync.dma_start(g_sb[:], g_t[:])

        ones1 = sb.tile([1, 128], f32, tag="ones1")
        nc.vector.memset(ones1[:], 1.0)
        ident = sb.tile([128, 128], f32, tag="ident")
        make_identity(nc, ident)

        inv = sb.tile([128, 6], f32, tag="inv")
        wn = sb.tile([128, 3 * NC65], f32, tag="wn")
        axn = sb.tile([128, 48], f32, tag="axn")
        ayn = sb.tile([128, 48], f32, tag="ayn")
        cn = sb.tile([128, 48], f32, tag="cn")
        tmp = sb.tile([128, 48], f32, tag="tmp")
        b2n = sb.tile([128, 3], f32, tag="b2n")
        w2t = sb.tile([HD, NPAD], mmdt, tag="w2t")
        axm = sb.tile([128, 48], mmdt, tag="axm")
        aym = sb.tile([128, 48], mmdt, tag="aym")
        cm = sb.tile([128, 48], mmdt, tag="cm")
        wpre = sb.tile([99, NPAD * HD], mmdt, tag="wpre")
        w2big = sb.tile([128, NBLK * 128], mmdt, tag="w2big")
        if prec == "bf16":
            nc.gpsimd.memset(w2big[:], 0.0)
        else:
            nc.gpsimd.memset(w2big[:].bitcast(f32), 0.0)

        with tc.tile_pool(name="setps", bufs=2, space="PSUM") as setps:
            for ci, (r0, nr) in enumerate([(0, 128), (128, 128), (256, 48)]):
                nc.sync.dma_start(posn[0:nr, 4 * ci:4 * ci + 4], pos_t[r0:r0 + nr, :])
                nc.vector.reciprocal(inv[0:nr, 2 * ci:2 * ci + 2],
                                     posn[0:nr, 4 * ci + 2:4 * ci + 4])
                # weights by-n = qT_chunk.T @ Wg + bg
                wps = setps.tile([128, NC65], f32, tag="wps")
                nc.tensor.matmul(wps[0:nr, :], qt0[:, r0:r0 + nr], wg0[:],
                                 start=True, stop=False)
                nc.tensor.matmul(wps[0:nr, :], qt1[:, r0:r0 + nr], wg1[:],
                                 start=False, stop=False)
                nc.tensor.matmul(wps[0:nr, :], ones1[:, 0:nr], bgr[:],
                                 start=False, stop=True)
                wnc = wn[0:nr, NC65 * ci:NC65 * ci + NC65]
                nc.vector.tensor_copy(wnc, wps[0:nr, :])

                axc = axn[0:nr, 16 * ci:16 * ci + 16]
                ayc = ayn[0:nr, 16 * ci:16 * ci + 16]
                cc = cn[0:nr, 16 * ci:16 * ci + 16]
                tc_ = tmp[0:nr, 16 * ci:16 * ci + 16]
                ibw = inv[0:nr, 2 * ci:2 * ci + 1]
                ibh = inv[0:nr, 2 * ci + 1:2 * ci + 2]
                cx = posn[0:nr, 4 * ci:4 * ci + 1]
                cy = posn[0:nr, 4 * ci + 1:4 * ci + 2]
                nc.vector.tensor_scalar(axc, wnc[:, 0:16], ibw, None, AO.mult)
                nc.vector.tensor_scalar(ayc, wnc[:, 16:32], ibh, None, AO.mult)
                nc.vector.tensor_scalar(tc_, axc, cx, None, AO.mult)
                nc.vector.tensor_sub(cc, wnc[:, 32:48], tc_)
                nc.vector.tensor_scalar(tc_, ayc, cy, None, AO.mult)
                nc.vector.tensor_sub(cc, cc, tc_)
                axmc = axm[0:nr, 16 * ci:16 * ci + 16]
                aymc = aym[0:nr, 16 * ci:16 * ci + 16]
                cmc = cm[0:nr, 16 * ci:16 * ci + 16]
                nc.vector.tensor_copy(axmc, axc)
                nc.vector.tensor_copy(aymc, ayc)
                nc.vector.tensor_copy(cmc, cc)
                nc.scalar.copy(b2n[0:nr, ci:ci + 1], wnc[:, 64:65])

                # w2 (by-n) -> transposed [16, n]
                wtp = setps.tile([HD, 128], f32, tag="wtp")
                nc.tensor.transpose(wtp[:, 0:nr], wnc[:, 48:64], ident[0:nr, 0:nr])
                nc.vector.tensor_copy(w2t[:, r0:r0 + nr], wtp[:, 0:nr])

                # Wpre rows: partition->free flatten  [nr, 16] -> [1, nr*16]
                for r, src_ap in ((0, axmc), (1, aymc), (2, cmc)):
                    dst = wpre[r:r + 1, HD * r0: HD * r0 + HD * nr]
                    nc.scalar.dma_start(
                        dst.rearrange("p (n k) -> p n k", k=HD), src_ap)

            for s in range(1, 4):
                nc.scalar.dma_start(wpre[32 * s:32 * s + 3, :], wpre[0:3, :])

            # w2big: zero-padded block-diagonal lhsT per block
            # col(B, m) = 128*B + 8*(B%16) + m ; partition rows 16m..16m+16
            # B = 16g + b -> col = 2048*g + 136*b + m
            for m in range(8):
                t = w2big[16 * m:16 * m + 16, 0:1].tensor
                ts_ = w2t[:, 0:1].tensor
                for g0, nb in ((0, 16), (1, 16), (2, 6)):
                    dst = bass.AP(t, (16 * m) * (NBLK * 128) + 2048 * g0 + m,
                                  [[NBLK * 128, 16], [136, nb]])
                    src = bass.AP(ts_, 128 * g0 + m,
                                  [[NPAD, 16], [8, nb]])
                    nc.sync.dma_start(dst, src)

        # ---------------- main loop ----------------
        pre_pool = ctx.enter_context(tc.tile_pool(name="prep", bufs=4, space="PSUM"))
        out_pool = ctx.enter_context(tc.tile_pool(name="outp", bufs=4, space="PSUM"))
        act_pool = ctx.enter_context(tc.tile_pool(name="actp", bufs=10))
        osb_pool = ctx.enter_context(tc.tile_pool(name="osbp", bufs=3))

        rl = 0  # relu parity for ACT/DVE balance
        for (b0, nblk, nrows) in GROUPS:
            gbase = 8 * b0
            for cg in range(NCH // CGRP):
                outs = [out_pool.tile([128, CH], f32, tag="ops", name=f"ops{cg}_{i}")
                        for i in range(CGRP)]
                pending = None  # (b, acts) awaiting contract, one block behind

                def flush(pending):
                    b, acts = pending
                    Bb = b0 + b
                    for ci in range(CGRP):
                        nc.tensor.matmul(outs[ci][0:nrows, :],
                                         w2big[:, 128 * Bb:128 * Bb + 128][:, 0:nrows],
                                         acts[ci][:],
                                         start=(b == 0), stop=(b == nblk - 1),
                                         skip_group_check=True)

                for b in range(nblk):
                    Bb = b0 + b
                    acts = []
                    for ci in range(CGRP):
                        c = CGRP * cg + ci
                        pre = pre_pool.tile([128, CH], f32, tag="pre")
                        nc.tensor.matmul(pre[:],
                                         wpre[32 * ci:32 * ci + 3,
                                              128 * Bb:128 * Bb + 128],
                                         g_sb[32 * ci:32 * ci + 3,
                                              CH * c:CH * c + CH],
                                         start=True, stop=True,
                                         tile_position=(32 * ci, 0))
                        act = act_pool.tile([128, CH], mmdt, tag="act")
                        if rl % 2 == 0:
                            nc.scalar.activation(act[:], pre[:], Relu)
                        else:
                            nc.vector.tensor_scalar(act[:], pre[:], 0.0, None, AO.max)
                        rl += 1
                        acts.append(act)
                    if pending is not None:
                        flush(pending)
                    pending = (b, acts)
                flush(pending)
                for ci in range(CGRP):
                    c = CGRP * cg + ci
                    osb = osb_pool.tile([128, CH], f32, tag="osb")
                    gi = b0 // 16
                    nc.scalar.add(osb[0:nrows, :], outs[ci][0:nrows, :],
                                  b2n[0:nrows, gi:gi + 1])
                    nc.sync.dma_start(out_t[gbase:gbase + nrows, CH * c:CH * c + CH],
                                      osb[0:nrows, :])

    _split_waits(nc)
    return nc


def _get_nc():
    if "nc" not in _CACHE:
        _CACHE["nc"] = _build()
    return _CACHE["nc"]


def _prepare_in_maps(pos, queries, Wg, bg):
    pos_f = np.asarray(pos, np.float32).reshape(-1, 4)
    q_f = np.asarray(queries, np.float32).reshape(-1, DIM)
    Wg = np.ascontiguousarray(np.asarray(Wg, np.float32))
    bg = np.ascontiguousarray(np.asarray(bg, np.float32).reshape(1, 4 * HD + 1))

    xs = (np.arange(W, dtype=np.float32) + 0.5) / W
    ys = (np.arange(H, dtype=np.float32) + 0.5) / H
    g3 = np.stack([np.tile(xs, H), np.repeat(ys, W),
                   np.ones(NPIX, np.float32)]).astype(np.float32)
    gx = np.zeros((99, NPIX), np.float32)
    for r in range(4):
        gx[32 * r:32 * r + 3] = g3
    if PREC == "bf16":
        import ml_dtypes
        gx = gx.astype(ml_dtypes.bfloat16)
    gx = np.ascontiguousarray(gx)

    in_maps = []
    for ci in range(NCORE):
        rows = slice(NPER * ci, NPER * (ci + 1))
        qT = np.zeros((DIM, NPAD), np.float32)
        qT[:, :NPER] = q_f[rows].T
        posn = np.tile(np.array([0.5, 0.5, 1.0, 1.0], np.float32), (NPAD, 1))
        posn[:NPER] = pos_f[rows]
        in_maps.append({"qT": qT, "posn": posn, "wg": Wg, "bgr": bg, "gx": gx})
    return in_maps


def kernel(pos, queries, Wg, bg, size):
    from concourse.bass_utils import run_bass_kernel_spmd

    assert int(size) == 64
    in_maps = _prepare_in_maps(pos, queries, Wg, bg)
    res = run_bass_kernel_spmd(_get_nc(), in_maps, list(range(NCORE)), trace=False)
    outs = [res.results[i]["out"][:NPER].reshape(NPER, H, W) for i in range(NCORE)]
    return np.concatenate(outs, 0).reshape(B_, Q_, H, W).astype(np.float32)


# revision 12
# speedup vs baseline: 2.3035x; 1.0425x over previous
"""Trainium2 Bass kernel for nn_PosMLP (box-relative position MLP).

Math (per query n):
  weights = q @ Wg + bg                       [N, 65]
  w1x = weights[:, 0:16], w1y = weights[:, 16:32]
  b1  = weights[:, 32:48], w2  = weights[:, 48:64], b2 = weights[:, 64]
  rel_x[j] = (xs[j] - cx)/bw ; rel_y[i] = (ys[i] - cy)/bh
  pre[i,j,k] = rel_x[j]*w1x[k] + rel_y[i]*w1y[k] + b1[k]
  out[i,j] = sum_k relu(pre)[i,j,k] * w2[k] + b2           (64x64 image)

Device mapping (per core, N=300 queries padded to 304 = 38 blocks of 8):
  - fold pos into per-(n,k) affine coeffs: ax = w1x/bw, ay = w1y/bh,
    c = b1 - cx*ax - cy*ay, so pre = xs[j]*ax + ys[i]*ay + c.
  - create: PE matmul K=3: lhsT = Wpre[3, 128(n,k)] per block,
    rhs = G[3, 512 pixels] (xs/ys/1 rows) -> psum pre[128, 512].
  - relu: ACT/DVE psum->SBUF.
  - contract: PE matmul K=128 with zero-padded block-diagonal
    lhsT[128(n,k), 128 n-cols]; 16 blocks accumulate into ONE psum bank
    -> dense [128 n, 512 pix] output; + b2 via per-partition ts-add.
All matmuls in float32r (full-speed fp32 path, ~3e-4 rel err).
"""
import numpy as np
from contextlib import ExitStack

# ---------------- problem constants (hardcoded per spec) ----------------
B_, Q_, DIM = 8, 300, 256
HD = 16            # hidden_dim
H = W = 64
NPIX = H * W       # 4096
CH = 512           # pixels per chunk
NCH = NPIX // CH   # 8
NCORE = 8
NPER = (B_ * Q_) // NCORE     # 300 queries per core
NPAD = 304                    # padded to 38 blocks of 8
NBLK = NPAD // 8              # 38
# psum groups: 16 blocks = 128 n rows per group (last group 6 blocks / 48 rows)
GROUPS = [(0, 16, 128), (16, 16, 128), (32, 6, 48)]
CGRP = 4                      # chunks per chunk-group (psum out banks)

_CACHE = {}


def _install_patches():
    """Shims for this container: walrus here rejects >1 sem-wait per
    instruction; split extra waits onto same-engine NOP carriers."""
    import bass_rust
    import concourse.tile as tile

    def _drain_and_barrier(self, tick_clock, wait_clock):
        nc = self.nc
        probe = nc.sync.nop(nofuse=True, hint="tile_drain_probe")
        wait_clock.add_sem_waits(
            probe.ins, bass_rust.ScopedClock({None: tick_clock.global_clock})
        )
        waits = list(probe.ins.sync_info.on_wait)
        probe.ins.sync_info.on_wait = []
        for i, w in enumerate(waits):
            carrier = nc.sync.nop(nofuse=True, hint=f"tile_drain_wait{i}")
            si = carrier.ins.sync_info
            if si is None:
                si = bass_rust.SyncInfo(on_wait=[], on_update=[])
                carrier.ins.sync_info = si
            si.on_wait = [w]
        nc.sync.drain()
        nc.all_engine_barrier()
        assert self.sems is not None
        popped = nc._tile_sem_poison_stack.pop()
        assert popped is self._sem_poison
        nc.clear_and_free_semaphores(list(self.sems.allocated().values()))
        nc.all_engine_barrier()

    tile.TileContext._drain_and_barrier = _drain_and_barrier


def _split_waits(nc, max_waits=1):
    import concourse.mybir as mybir

    for f in nc.m.functions:
        for bb in f.blocks:
            insts = bb.instructions
            out = []
            changed = False
            for inst in insts:
                si = inst.sync_info
                waits = list(si.on_wait) if si is not None else []
                if len(waits) > max_waits:
                    keep = waits[:max_waits]
                    rest = waits[max_waits:]
                    for j in range(0, len(rest), max_waits):
                        out.append(
                            mybir.InstNoOp(
                                name=f"{inst.name}_wsplit{j}",
                                engine=inst.engine,
                                bass_nofuse=True,
                                sync_info=mybir.SyncInfo(
                                    on_wait=rest[j : j + max_waits], on_update=[]
                                ),
                            )
                        )
                    si.on_wait = keep
                    changed = True
                out.append(inst)
            if changed:
                insts[:] = out


PREC = "bf16"   # "bf16" or "f32r" for the create/contract matmuls


def _build(prec=None):
    import concourse.bass as bass
    import concourse.tile as tile
    import concourse.mybir as mybir
    from concourse.masks import make_identity

    if prec is None:
        prec = PREC
    _install_patches()
    f32 = mybir.dt.float32
    f32r = mybir.dt.float32r
    mmdt = mybir.dt.bfloat16 if prec == "bf16" else f32r
    AO = mybir.AluOpType
    Relu = mybir.ActivationFunctionType.Relu

    nc = bass.Bass()
    qT_t = nc.dram_tensor("qT", (DIM, NPAD), f32, kind="ExternalInput")
    pos_t = nc.dram_tensor("posn", (NPAD, 4), f32, kind="ExternalInput")
    wg_t = nc.dram_tensor("wg", (DIM, 4 * HD + 1), f32, kind="ExternalInput")
    bg_t = nc.dram_tensor("bgr", (1, 4 * HD + 1), f32, kind="ExternalInput")
    g_t = nc.dram_tensor("gx", (99, NPIX), mmdt, kind="ExternalInput")
    out_t = nc.dram_tensor("out", (NPAD, NPIX), f32, kind="ExternalOutput")

    NC65 = 4 * HD + 1

    with ExitStack() as ctx:
        tc = ctx.enter_context(tile.TileContext(nc))
        sb = ctx.enter_context(tc.tile_pool(name="sb", bufs=1))

        # ---------------- load inputs ----------------
        qt0 = sb.tile([128, NPAD], f32, tag="qt0")
        nc.sync.dma_start(qt0[:], qT_t[0:128, :])
        qt1 = sb.tile([128, NPAD], f32, tag="qt1")
        nc.sync.dma_start(qt1[:], qT_t[128:256, :])
        wg0 = sb.tile([128, NC65], f32, tag="wg0")
        nc.sync.dma_start(wg0[:], wg_t[0:128, :])
        wg1 = sb.tile([128, NC65], f32, tag="wg1")
        nc.sync.dma_start(wg1[:], wg_t[128:256, :])
        bgr = sb.tile([1, NC65], f32, tag="bgr")
        nc.sync.dma_start(bgr[:], bg_t[:])
        posn = sb.tile([128, 12], f32, tag="posn")
        g_sb = sb.tile([99, NPIX], mmdt, tag="gsb")
        nc.sync.dma_start(g_sb[:], g_t[:])

        ones1 = sb.tile([1, 128], f32, tag="ones1")
        nc.vector.memset(ones1[:], 1.0)
        ident = sb.tile([128, 128], f32, tag="ident")
        make_identity(nc, ident)

        inv = sb.tile([128, 6], f32, tag="inv")
        wn = sb.tile([128, 3 * NC65], f32, tag="wn")
        axn = sb.tile([128, 48], f32, tag="axn")
        ayn = sb.tile([128, 48], f32, tag="ayn")
        cn = sb.tile([128, 48], f32, tag="cn")
        tmp = sb.tile([128, 48], f32, tag="tmp")
        b2n = sb.tile([128, 3], f32, tag="b2n")
        w2t = sb.tile([HD, NPAD], mmdt, tag="w2t")
        axm = sb.tile([128, 48], mmdt, tag="axm")
        aym = sb.tile([128, 48], mmdt, tag="aym")
        cm = sb.tile([128, 48], mmdt, tag="cm")
        wpre_c = [sb.tile([99, HD * nrc], mmdt, tag=f"wpre{i}", name=f"wpre{i}")
                  for i, nrc in enumerate((128, 128, 48))]
        w2big_g = []
        for i, nbg in enumerate((16, 16, 6)):
            wbt = sb.tile([128, nbg * 128], mmdt, tag=f"w2big{i}", name=f"w2big{i}")
            if prec == "bf16":
                nc.gpsimd.memset(wbt[:], 0.0)
            else:
                nc.gpsimd.memset(wbt[:].bitcast(f32), 0.0)
            w2big_g.append(wbt)

        with tc.tile_pool(name="setps", bufs=2, space="PSUM") as setps:
            for ci, (r0, nr) in enumerate([(0, 128), (128, 128), (256, 48)]):
                nc.sync.dma_start(posn[0:nr, 4 * ci:4 * ci + 4], pos_t[r0:r0 + nr, :])
                nc.vector.reciprocal(inv[0:nr, 2 * ci:2 * ci + 2],
                                     posn[0:nr, 4 * ci + 2:4 * ci + 4])
                # weights by-n = qT_chunk.T @ Wg + bg
                wps = setps.tile([128, NC65], f32, tag="wps")
                nc.tensor.matmul(wps[0:nr, :], qt0[:, r0:r0 + nr], wg0[:],
                                 start=True, stop=False)
                nc.tensor.matmul(wps[0:nr, :], qt1[:, r0:r0 + nr], wg1[:],
                                 start=False, stop=False)
                nc.tensor.matmul(wps[0:nr, :], ones1[:, 0:nr], bgr[:],
                                 start=False, stop=True)
                wnc = wn[0:nr, NC65 * ci:NC65 * ci + NC65]
                nc.vector.tensor_copy(wnc, wps[0:nr, :])

                axc = axn[0:nr, 16 * ci:16 * ci + 16]
                ayc = ayn[0:nr, 16 * ci:16 * ci + 16]
                cc = cn[0:nr, 16 * ci:16 * ci + 16]
                tc_ = tmp[0:nr, 16 * ci:16 * ci + 16]
                ibw = inv[0:nr, 2 * ci:2 * ci + 1]
                ibh = inv[0:nr, 2 * ci + 1:2 * ci + 2]
                cx = posn[0:nr, 4 * ci:4 * ci + 1]
                cy = posn[0:nr, 4 * ci + 1:4 * ci + 2]
                nc.vector.tensor_scalar(axc, wnc[:, 0:16], ibw, None, AO.mult)
                nc.vector.tensor_scalar(ayc, wnc[:, 16:32], ibh, None, AO.mult)
                nc.vector.tensor_scalar(tc_, axc, cx, None, AO.mult)
                nc.vector.tensor_sub(cc, wnc[:, 32:48], tc_)
                nc.vector.tensor_scalar(tc_, ayc, cy, None, AO.mult)
                nc.vector.tensor_sub(cc, cc, tc_)
                axmc = axm[0:nr, 16 * ci:16 * ci + 16]
                aymc = aym[0:nr, 16 * ci:16 * ci + 16]
                cmc = cm[0:nr, 16 * ci:16 * ci + 16]
                nc.vector.tensor_copy(axmc, axc)
                nc.vector.tensor_copy(aymc, ayc)
                nc.vector.tensor_copy(cmc, cc)
                nc.scalar.copy(b2n[0:nr, ci:ci + 1], wnc[:, 64:65])

                # w2 (by-n) -> transposed [16, n]
                wtp = setps.tile([HD, 128], f32, tag="wtp")
                nc.tensor.transpose(wtp[:, 0:nr], wnc[:, 48:64], ident[0:nr, 0:nr])
                nc.vector.tensor_copy(w2t[:, r0:r0 + nr], wtp[:, 0:nr])

                # Wpre rows: partition->free flatten  [nr, 16] -> [1, nr*16]
                for r, src_ap in ((0, axmc), (1, aymc), (2, cmc)):
                    for s in range(4):
                        dst = wpre_c[ci][32 * s + r:32 * s + r + 1, :]
                        eng = nc.scalar if (s + r) % 2 else nc.sync
                        eng.dma_start(
                            dst.rearrange("p (n k) -> p n k", k=HD), src_ap)

            # w2big: zero-padded block-diagonal lhsT per block
            # within group g: col(b, m) = 128*b + 8*b + m = 136*b + m
            for m in range(8):
                ts_ = w2t[:, 0:1].tensor
                for g0, nb in ((0, 16), (1, 16), (2, 6)):
                    t = w2big_g[g0][0:1, 0:1].tensor
                    row = w2big_g[g0].shape[1]
                    dst = bass.AP(t, (16 * m) * row + m, [[row, 16], [136, nb]])
                    src = bass.AP(ts_, 128 * g0 + m, [[NPAD, 16], [8, nb]])
                    eng = nc.scalar if m % 2 else nc.sync
                    eng.dma_start(dst, src)

        # ---------------- main loop ----------------
        pre_pool = ctx.enter_context(tc.tile_pool(name="prep", bufs=4, space="PSUM"))
        out_pool = ctx.enter_context(tc.tile_pool(name="outp", bufs=4, space="PSUM"))
        act_pool = ctx.enter_context(tc.tile_pool(name="actp", bufs=10))
        osb_pool = ctx.enter_context(tc.tile_pool(name="osbp", bufs=3))

        rl = 0  # relu parity for ACT/DVE balance
        for (b0, nblk, nrows) in GROUPS:
            gbase = 8 * b0
            for cg in range(NCH // CGRP):
                outs = [out_pool.tile([128, CH], f32, tag="ops", name=f"ops{cg}_{i}")
                        for i in range(CGRP)]
                pending = None  # (b, acts) awaiting contract, one block behind

                def flush(pending):
                    b, acts = pending
                    Bb = b0 + b
                    for ci in range(CGRP):
                        nc.tensor.matmul(outs[ci][0:nrows, :],
                                         w2big_g[Bb // 16][:, 128 * (Bb % 16):
                                                           128 * (Bb % 16) + nrows],
                                         acts[ci][:],
                                         start=(b == 0), stop=(b == nblk - 1),
                                         skip_group_check=True)

                for b in range(nblk):
                    Bb = b0 + b
                    acts = []
                    for ci in range(CGRP):
                        c = CGRP * cg + ci
                        pre = pre_pool.tile([128, CH], f32, tag="pre")
                        nc.tensor.matmul(pre[:],
                                         wpre_c[Bb // 16][32 * ci:32 * ci + 3,
                                              128 * (Bb % 16):128 * (Bb % 16) + 128],
                                         g_sb[32 * ci:32 * ci + 3,
                                              CH * c:CH * c + CH],
                                         start=True, stop=True,
                                         tile_position=(32 * ci, 0))
                        act = act_pool.tile([128, CH], mmdt, tag="act")
                        if rl % 2 == 0:
                            nc.scalar.activation(act[:], pre[:], Relu)
                        else:
                            nc.vector.tensor_scalar(act[:], pre[:], 0.0, None, AO.max)
                        rl += 1
                        acts.append(act)
                    if pending is not None:
                        flush(pending)
                    pending = (b, acts)
                flush(pending)
                for ci in range(CGRP):
                    c = CGRP * cg + ci
                    osb = osb_pool.tile([128, CH], f32, tag="osb")
                    gi = b0 // 16
                    nc.scalar.add(osb[0:nrows, :], outs[ci][0:nrows, :],
                                  b2n[0:nrows, gi:gi + 1])
                    nc.sync.dma_start(out_t[gbase:gbase + nrows, CH * c:CH * c + CH],
                                      osb[0:nrows, :])

    _split_waits(nc)
    return nc


def _get_nc():
    if "nc" not in _CACHE:
        _CACHE["nc"] = _build()
    return _CACHE["nc"]


def _prepare_in_maps(pos, queries, Wg, bg):
    pos_f = np.asarray(pos, np.float32).reshape(-1, 4)
    q_f = np.asarray(queries, np.float32).reshape(-1, DIM)
    Wg = np.ascontiguousarray(np.asarray(Wg, np.float32))
    bg = np.ascontiguousarray(np.asarray(bg, np.float32).reshape(1, 4 * HD + 1))

    xs = (np.arange(W, dtype=np.float32) + 0.5) / W
    ys = (np.arange(H, dtype=np.float32) + 0.5) / H
    g3 = np.stack([np.tile(xs, H), np.repeat(ys, W),
                   np.ones(NPIX, np.float32)]).astype(np.float32)
    gx = np.zeros((99, NPIX), np.float32)
    for r in range(4):
        gx[32 * r:32 * r + 3] = g3
    if PREC == "bf16":
        import ml_dtypes
        gx = gx.astype(ml_dtypes.bfloat16)
    gx = np.ascontiguousarray(gx)

    in_maps = []
    for ci in range(NCORE):
        rows = slice(NPER * ci, NPER * (ci + 1))
        qT = np.zeros((DIM, NPAD), np.float32)
        qT[:, :NPER] = q_f[rows].T
        posn = np.tile(np.array([0.5, 0.5, 1.0, 1.0], np.float32), (NPAD, 1))
        posn[:NPER] = pos_f[rows]
        in_maps.append({"qT": qT, "posn": posn, "wg": Wg, "bgr": bg, "gx": gx})
    return in_maps


def kernel(pos, queries, Wg, bg, size):
    from concourse.bass_utils import run_bass_kernel_spmd

    assert int(size) == 64
    in_maps = _prepare_in_maps(pos, queries, Wg, bg)
    res = run_bass_kernel_spmd(_get_nc(), in_maps, list(range(NCORE)), trace=False)
    outs = [res.results[i]["out"][:NPER].reshape(NPER, H, W) for i in range(NCORE)]
    return np.concatenate(outs, 0).reshape(B_, Q_, H, W).astype(np.float32)
